# revision 2
# baseline (speedup 1.0000x reference)
"""Trainium2 Bass kernel for nn_ActorNetwork (gnn_message_passing).

Mathematical collapse (verified vs reference to ~2.5e-8 rel): the reference
broadcasts edge_index as ``broadcast_to(ei[None], (B,2,E)).reshape(2,-1)``,
making row == col elementwise -> every edge is a self-loop and the GCN
normalization cancels exactly: ``gcn_conv(x, W, b) == x @ W + b``.  The
network is two dense layers + softmax over nodes, plus a per-(node,k)
2-layer MLP + softmax over k.  ``edge_index`` never ships to the device.
Scalar biases bfc / bc2 are constant softmax shifts and cancel.

Device strategy (data-parallel over batch, core i = graphs 2i, 2i+1):
 - Host pre-transposes to feature-major so every DMA is contiguous; all
   per-group inputs are packed into ONE [128, 7000] fp8 blob per group so
   each group is a single ~896 KB dma_start (good DMA efficiency, few
   dispatches).
 - Inputs fp8e4m3, intermediates bf16, accumulation f32, output bf16.
 - 20 chunks of 500 nodes, 4 chunks per PSUM group (partition slots
   0/32/64/96) so elementwise engines see FD=500, 128-partition tiles.
 - 1/D via the single-instruction DVE reciprocal_approx_fast (~18 bits).
 - Elementwise work split ACT/DVE to balance engine busy time.
"""

import numpy as np

B, N, F, K, FC = 16, 5000, 128, 10, 32
NCORES = 8
GPC = B // NCORES          # graphs per core = 2
M = GPC * N                # nodes per core = 10000
CH = 500                   # chunk size (nodes)
NCHUNK = M // CH           # 20
GRP = 4                    # chunks per PSUM group
NGRP = NCHUNK // GRP       # 5
CPG = N // CH              # chunks per graph = 10
GW = 7000                  # blob columns per group: 2000+2000+2000+1000

# column maps for the three packed constant tensors
_W8, _WBF, _W32 = {}, {}, {}
def _mk(dct, *spec):
    off = 0
    for name, width in spec:
        dct[name] = (off, off + width)
        off += width
    return off
_NW8 = _mk(_W8, ("w1p", 32), ("wc1b", 64), ("wc1b2p", 64), ("pad8", 352))
_NWB = _mk(_WBF, ("w2b", 64), ("wfcb", 4), ("wc2a", 32), ("wc2b", 32),
           ("kmask", 4), ("padb", 120))
_NW32 = _mk(_W32, ("bmask", 128), ("smsk", 2 * NGRP), ("smap", 4 * NGRP),
            ("b1r", 1), ("b2r", 1), ("bc1r", 1))


def _pack_consts(W1, b1, W2, b2, Wfc, Wc1, bc1, Wc2):
    import ml_dtypes
    w8 = np.zeros((128, _NW8), np.float32)
    wb = np.zeros((128, _NWB), np.float32)
    w3 = np.zeros((128, _NW32), np.float32)

    lo = _W8["w1p"][0]
    w8[:, lo:lo + 16] = W1
    lo = _W8["wc1b"][0]
    for a in range(4):
        w8[32 * a:32 * a + 32, lo + 16 * a:lo + 16 * a + 16] = Wc1
    lo = _W8["wc1b2p"][0]
    for s in range(2):          # chunk parity within a pair
        for t in range(2):      # k = 8 + t
            r = 64 * s + 32 * t
            w8[r:r + 32, lo + 32 * s + 16 * t:lo + 32 * s + 16 * t + 16] = Wc1

    lo = _WBF["w2b"][0]
    for j in range(4):
        wb[32 * j:32 * j + 16, lo + 16 * j:lo + 16 * j + 16] = W2
    lo = _WBF["wfcb"][0]
    for j in range(4):
        wb[16 * j:16 * j + 16, lo + j] = Wfc[:, 0]
    lo = _WBF["wc2a"][0]
    for k in range(8):
        wb[16 * k:16 * k + 16, lo + k] = Wc2[:, 0]
    lo = _WBF["wc2b"][0]
    for j in range(4):
        for t in range(2):
            r = 32 * j + 16 * t
            wb[r:r + 16, lo + 8 + t] = Wc2[:, 0]
    lo = _WBF["kmask"][0]
    for j in range(4):
        wb[32 * j:32 * j + 10, lo + j] = 1.0

    lo = _W32["bmask"][0]
    for j in range(4):
        w3[j, lo + 32 * j:lo + 32 * j + 10] = 1.0
    lo = _W32["smsk"][0]
    for c in range(NGRP):
        for j in range(GRP):
            g = (GRP * c + j) // CPG
            w3[j, lo + 2 * c + g] = 1.0
    lo = _W32["smap"][0]
    for c in range(NGRP):
        for j in range(GRP):
            g = (GRP * c + j) // CPG
            w3[g, lo + 4 * c + j] = 1.0
    lo = _W32["b1r"][0]
    for j in range(4):
        w3[32 * j:32 * j + 16, lo] = b1
    lo = _W32["b2r"][0]
    for j in range(4):
        w3[16 * j:16 * j + 16, lo] = b2
    lo = _W32["bc1r"][0]
    for s in range(8):
        w3[16 * s:16 * s + 16, lo] = bc1
    return (w8.astype(ml_dtypes.float8_e4m3), wb.astype(ml_dtypes.bfloat16),
            w3)


_CACHED = None


def _build():
    """Build + bacc-compile the per-core Bass graph (same on all 8 cores)."""
    from contextlib import ExitStack

    import concourse.tile as tile
    from concourse import bacc, mybir

    f32 = mybir.dt.float32
    bf16 = mybir.dt.bfloat16
    f8 = mybir.dt.float8e4
    AF = mybir.ActivationFunctionType
    ALU = mybir.AluOpType

    import concourse.bacc as bacc_mod
    _orig_gat = bacc_mod.get_activation_tables

    def _gat_one_set(arch):
        # keep every entry (act_func_set_id is positional) but empty all
        # sets except the one that covers Relu+Exp, forcing its choice
        t = _orig_gat(arch)
        if "natural_log_exp_and_others" not in t:
            return t
        return {k: (v if k == "natural_log_exp_and_others" else set())
                for k, v in t.items()}

    bacc_mod.get_activation_tables = _gat_one_set

    nc = bacc.Bacc("TRN2", target_bir_lowering=False, debug=False,
                   num_devices=NCORES)

    xa_p = nc.dram_tensor("xa", [128, NGRP * GW], f8, kind="ExternalInput").ap()
    w8_p = nc.dram_tensor("w8", [128, _NW8], f8, kind="ExternalInput").ap()
    wb_p = nc.dram_tensor("wb", [128, _NWB], bf16, kind="ExternalInput").ap()
    w3_p = nc.dram_tensor("w3", [128, _NW32], f32, kind="ExternalInput").ap()
    out_p = nc.dram_tensor("out", [128, NGRP * CH], bf16,
                       kind="ExternalOutput").ap()

    with tile.TileContext(nc) as tc, ExitStack() as ctx:
        wpool = ctx.enter_context(tc.tile_pool(name="wc", bufs=1))
        wt8 = wpool.tile([128, _NW8], f8, tag="wt8")
        nc.sync.dma_start(out=wt8[:], in_=w8_p[:])
        wtb = wpool.tile([128, _NWB], bf16, tag="wtb")
        wt3 = wpool.tile([128, _NW32], f32, tag="wt3")

        def w8s(name, rows=128):
            lo, hi = _W8[name]
            return wt8[0:rows, lo:hi]

        def wbs(name, rows=128):
            lo, hi = _WBF[name]
            return wtb[0:rows, lo:hi]

        def w3s(name, rows=128):
            lo, hi = _W32[name]
            return wt3[0:rows, lo:hi]

        sb = ctx.enter_context(tc.tile_pool(name="sb", bufs=2))
        ldp = ctx.enter_context(tc.tile_pool(name="ld", bufs=3))
        elp = ctx.enter_context(tc.tile_pool(name="el", bufs=NGRP))
        eyp = ctx.enter_context(tc.tile_pool(name="ey", bufs=NGRP))
        dvp = ctx.enter_context(tc.tile_pool(name="dv", bufs=NGRP))
        accp = ctx.enter_context(tc.tile_pool(name="acc", bufs=1))
        m4p = ctx.enter_context(tc.tile_pool(name="m4p", bufs=NGRP))
        outp = ctx.enter_context(tc.tile_pool(name="outs", bufs=3))
        bsp = ctx.enter_context(tc.tile_pool(name="bsp", bufs=NGRP))
        el_tiles, ey_tiles, m4_tiles = [], [], []
        accs = accp.tile([GRP, NGRP], f32)

        # ---------------- main loop: node + col(A) per group ----------
        with tc.tile_pool(name="h1pp", bufs=1, space="PSUM") as h1pp, \
             tc.tile_pool(name="h2lp", bufs=1, space="PSUM") as h2lp, \
             tc.tile_pool(name="h01pp", bufs=2, space="PSUM") as h01pp, \
             tc.tile_pool(name="h2cpp", bufs=1, space="PSUM") as h2cpp, \
             tc.tile_pool(name="ypp", bufs=1, space="PSUM") as ypp, \
             tc.tile_pool(name="l4pp", bufs=1, space="PSUM") as l4pp, \
             tc.tile_pool(name="d4pp", bufs=1, space="PSUM") as d4pp:
            for c in range(NGRP):
                w = GRP * CH
                blob = ldp.tile([128, GW], f8, tag="blob", name="blob")
                nc.sync.dma_start(out=blob[:], in_=xa_p[:, GW * c:GW * (c + 1)])
                xg2 = blob[:, 0:2000]
                c0 = blob[:, 2000:4000]
                c1 = blob[:, 4000:6000]
                c2t = blob[:, 6000:7000]
                if c == 0:
                    nc.sync.dma_start(out=wt3[:], in_=w3_p[:])
                    nc.sync.dma_start(out=wtb[:], in_=wb_p[:])
                    # pre-load the ACT spline table set (relu/exp both in
                    # natural_log_exp_and_others) with no DMA dependency
                    warm = sb.tile([1, 2], f32, tag="warm")
                    nc.vector.memset(warm[:, 0:1], 0.0)
                    nc.scalar.activation(warm[:, 1:2], warm[:, 0:1], AF.Exp)

                # --- node path ---
                h1p = h1pp.tile([128, 512], f32, tag="h1p", name="h1p")[:, 0:CH]
                for j in range(GRP):
                    nc.tensor.matmul(h1p[32 * j:32 * j + 32, :],
                                     lhsT=w8s("w1p"),
                                     rhs=xg2[:, CH * j:CH * (j + 1)],
                                     start=True, stop=True,
                                     tile_position=(0, 32 * j))
                h1s = sb.tile([128, CH], bf16, tag="h1s")
                nc.scalar.activation(h1s[:], h1p[:], AF.Relu, bias=w3s("b1r"))
                h2l = h2lp.tile([64, 512], f32, tag="h2l", name="h2l")[:, 0:CH]
                nc.tensor.matmul(h2l[:], lhsT=wbs("w2b"), rhs=h1s[:],
                                 start=True, stop=True)
                h2s = sb.tile([64, CH], bf16, tag="h2s")
                nc.scalar.activation(h2s[:], h2l[:], AF.Relu,
                                     bias=w3s("b2r", 64))
                l4p = l4pp.tile([GRP, 512], f32, tag="l4p",
                                name="l4p")[:, 0:CH]
                nc.tensor.matmul(l4p[:], lhsT=wbs("wfcb", 64),
                                 rhs=h2s[:], start=True, stop=True)
                el = elp.tile([GRP, CH], bf16, tag="el")
                nc.scalar.activation(el[:], l4p[:], AF.Exp,
                                     accum_out=accs[:, c:c + 1])

                # --- col path (softmax-independent part) ---
                h2cp = h2cpp.tile([128, 512], f32, tag="h2cp",
                                  name="h2cp")[:, 0:CH]
                for p in range(2):
                    nc.tensor.matmul(h2cp[64 * p:64 * p + 64, :],
                                     lhsT=w8s("wc1b2p"),
                                     rhs=c2t[:, CH * p:CH * (p + 1)],
                                     start=True, stop=True,
                                     tile_position=(0, 64 * p))
                h01s_tiles = []
                for j in range(GRP):
                    cs = slice(CH * j, CH * (j + 1))
                    h01p = h01pp.tile([128, 512], f32, tag="h01p",
                                      name="h01p")[:, 0:CH]
                    nc.tensor.matmul(h01p[0:64, :], lhsT=w8s("wc1b"),
                                     rhs=c0[:, cs], start=True, stop=True)
                    nc.tensor.matmul(h01p[64:128, :], lhsT=w8s("wc1b"),
                                     rhs=c1[:, cs], start=True, stop=True,
                                     tile_position=(0, 64))
                    h01s = sb.tile([128, CH], bf16, tag=f"h01s{j % 2}")
                    if j == 3:
                        nc.scalar.activation(h01s[:], h01p[:], AF.Relu,
                                             bias=w3s("bc1r"))
                    else:
                        nc.vector.tensor_scalar(h01s[:], h01p[:], w3s("bc1r"),
                                                0.0, ALU.add, ALU.max)
                    h01s_tiles.append(h01s)
                h2cs = sb.tile([128, CH], bf16, tag="h2cs")
                nc.scalar.activation(h2cs[:], h2cp[:], AF.Relu,
                                     bias=w3s("bc1r"))
                yp = ypp.tile([128, 512], f32, tag="yp", name="yp")[:, 0:CH]
                for j in range(GRP):
                    nc.tensor.matmul(yp[32 * j:32 * j + 32, :],
                                     lhsT=wbs("wc2a"), rhs=h01s_tiles[j][:],
                                     start=True, stop=False,
                                     skip_group_check=True,
                                     tile_position=(0, 32 * j))
                for j in range(GRP):
                    nc.tensor.matmul(yp[32 * j:32 * j + 32, :],
                                     lhsT=wtb[32 * j:32 * j + 32,
                                              slice(*_WBF["wc2b"])],
                                     rhs=h2cs[32 * j:32 * j + 32, :],
                                     start=False, stop=True,
                                     skip_group_check=True,
                                     tile_position=(32 * j, 32 * j))
                ey = eyp.tile([128, CH], bf16, tag="ey")
                nc.scalar.activation(ey[:], yp[:], AF.Exp)
                ey_tiles.append(ey)
                d4p = d4pp.tile([GRP, 512], f32, tag="ps", name="d4p")[:, 0:CH]
                nc.tensor.matmul(d4p[:], lhsT=wbs("kmask"), rhs=ey[:],
                                 start=True, stop=True)
                dinv = dvp.tile([GRP, CH], f32, tag="dinv")
                nc.vector.reciprocal_approx_fast(dinv[:], d4p[:])
                m4 = m4p.tile([GRP, CH], bf16, tag="m4")
                nc.vector.tensor_mul(m4[:], el[:], dinv[:])
                m4_tiles.append(m4)

            # ---------- finalize: softmax normalization + output ------
            sp = d4pp.tile([2, 512], f32, tag="ps", name="sp")[:, 0:1]
            lo = _W32["smsk"][0]
            for c in range(NGRP):
                nc.tensor.matmul(sp[:],
                                 lhsT=wt3[0:GRP, lo + 2 * c:lo + 2 * c + 2],
                                 rhs=accs[:, c:c + 1],
                                 start=(c == 0), stop=(c == NGRP - 1),
                                 skip_group_check=True)
            sinv = sb.tile([2, 1], f32, tag="sinv")
            nc.vector.reciprocal(sinv[:], sp[:])
            s4p = d4pp.tile([GRP, 512], f32, tag="ps", name="s4p")[:, 0:NGRP]
            lo = _W32["smap"][0]
            for c in range(NGRP):
                nc.tensor.matmul(s4p[:, c:c + 1],
                                 lhsT=wt3[0:2, lo + 4 * c:lo + 4 * c + 4],
                                 rhs=sinv[:], start=True, stop=True,
                                 skip_group_check=True)
            s4s = sb.tile([GRP, NGRP], f32, tag="s4s")
            nc.vector.tensor_copy(s4s[:], s4p[:])
            for c in range(NGRP):
                bst = bsp.tile([GRP, 128], bf16, tag="bs")
                nc.vector.tensor_scalar_mul(bst[:], w3s("bmask", GRP),
                                            s4s[:, c:c + 1])
                mbp = d4pp.tile([128, 512], f32, tag="ps",
                                name="mbp")[:, 0:CH]
                nc.tensor.matmul(mbp[:], lhsT=bst[:], rhs=m4_tiles[c][:],
                                 start=True, stop=True)
                ot = outp.tile([128, CH], bf16, tag="ot")
                nc.vector.tensor_mul(ot[:], ey_tiles[c][:], mbp[:])
                nc.gpsimd.dma_start(out=out_p[:, CH * c:CH * (c + 1)],
                                    in_=ot[:])

    nc.compile()
    bacc_mod.get_activation_tables = _orig_gat
    return nc


def _get_compiled():
    global _CACHED
    if _CACHED is None:
        _CACHED = _build()
    return _CACHED


def _prep_inputs(node_features, col_features, W1, b1, W2, b2, Wfc,
                 Wc1, bc1, Wc2):
    import ml_dtypes
    f8 = ml_dtypes.float8_e4m3
    nf = np.asarray(node_features, np.float32)
    cf = np.asarray(col_features, np.float32)
    xt = np.ascontiguousarray(
        nf.reshape(NCORES, GPC, N, F).transpose(0, 3, 1, 2)
        .reshape(NCORES, F, M)).astype(f8)
    ctf = np.ascontiguousarray(
        cf.reshape(NCORES, GPC, N, K, FC).transpose(0, 3, 4, 1, 2)
        .reshape(NCORES, K * FC, M)).astype(f8)
    # k = 8,9 rows, chunk-paired: [parity, 64 rows, pairs, 500] -> [128, M/2]
    c2 = np.ascontiguousarray(
        ctf[:, 256:320].reshape(NCORES, 64, M // (2 * CH), 2, CH)
        .transpose(0, 3, 1, 2, 4).reshape(NCORES, 128, M // 2))
    # pack per-group blobs: [xg2 | c0 | c1 | c2t] = [128, 7000] each
    xa = np.empty((NCORES, 128, NGRP * GW), f8)
    for c in range(NGRP):
        o = GW * c
        w = GRP * CH
        xa[:, :, o:o + 2000] = xt[:, :, w * c:w * (c + 1)]
        xa[:, :, o + 2000:o + 4000] = ctf[:, 0:128, w * c:w * (c + 1)]
        xa[:, :, o + 4000:o + 6000] = ctf[:, 128:256, w * c:w * (c + 1)]
        xa[:, :, o + 6000:o + 7000] = c2[:, :, w * c // 2:w * (c + 1) // 2]
    w8, wb, w3 = _pack_consts(
        np.asarray(W1, np.float32), np.asarray(b1, np.float32),
        np.asarray(W2, np.float32), np.asarray(b2, np.float32),
        np.asarray(Wfc, np.float32), np.asarray(Wc1, np.float32),
        np.asarray(bc1, np.float32), np.asarray(Wc2, np.float32))
    return xa, w8, wb, w3


def kernel(node_features, col_features, edge_index=None,
           W1=None, b1=None, W2=None, b2=None, Wfc=None, bfc=None,
           Wc1=None, bc1=None, Wc2=None, bc2=None, **_unused):
    from concourse.bass_utils import run_bass_kernel_spmd

    xa, w8, wb, w3 = _prep_inputs(node_features, col_features,
                                  W1, b1, W2, b2, Wfc, Wc1, bc1, Wc2)
    nc = _get_compiled()
    in_maps = [{"xa": xa[i], "w8": w8, "wb": wb, "w3": w3}
               for i in range(NCORES)]
    res = run_bass_kernel_spmd(nc, in_maps, core_ids=list(range(NCORES)))
    outs = np.stack([np.asarray(res.results[i]["out"], np.float32)
                     for i in range(NCORES)])
    # outs[i][32j+k, 500c+nn] = value for node 2000c+500j+nn, class k
    o = outs.reshape(NCORES, 4, 32, NGRP, CH)[:, :, 0:K]   # [i, j, k, c, nn]
    o = o.transpose(0, 3, 1, 4, 2)                         # [i, c, j, nn, k]
    out = o.reshape(NCORES, GPC, N, K).reshape(B, N * K)
    return np.ascontiguousarray(out)


# revision 13
# speedup vs baseline: 1.2116x; 1.2116x over previous
"""Trainium2 Bass kernel for nn_ActorNetwork (gnn_message_passing).

Mathematical collapse (verified vs reference to ~2.5e-8 rel): the reference
broadcasts edge_index as ``broadcast_to(ei[None], (B,2,E)).reshape(2,-1)``,
making row == col elementwise -> every edge is a self-loop and the GCN
normalization cancels exactly: ``gcn_conv(x, W, b) == x @ W + b``.  The
network is two dense layers + softmax over nodes, plus a per-(node,k)
2-layer MLP + softmax over k.  ``edge_index`` never ships to the device.
Scalar biases bfc / bc2 are constant softmax shifts and cancel.

Device strategy (data-parallel over batch, core i = graphs 2i, 2i+1):
 - Host pre-transposes to feature-major; per-group inputs are packed into
   one [128, 7000] fp8 blob per group, loaded in two dma_starts.
 - Chunk remap: group g holds chunks {g, 5+g, 10+g, 15+g} at rows j=0..3,
   so graph membership = j//2 for EVERY group.  The node-softmax finalize
   collapses to: per-group sp accumulation (interleaved), one reciprocal,
   one [4,1] broadcast matmul; 1/S folds into the m4 multiply.
 - l4p / d4p / el / dinv / m4 live on partitions 64-67 (tile_position col
   64) so l4p shares the h2l PSUM bank, freeing a bank for the finalize.
 - 1/D via single-instruction DVE reciprocal_approx_fast (~18 bits).
 - Inputs fp8e4m3, intermediates bf16, accumulation f32, output bf16.
"""

import numpy as np

B, N, F, K, FC = 16, 5000, 128, 10, 32
NCORES = 8
GPC = B // NCORES          # graphs per core = 2
M = GPC * N                # nodes per core = 10000
CH = 500                   # chunk size (nodes)
NCHUNK = M // CH           # 20
GRP = 4                    # chunks per PSUM group
NGRP = NCHUNK // GRP       # 5
GW = 7000                  # blob columns per group: 2000+2000+2000+1000

# column maps for the three packed constant tensors
_W8, _WBF, _W32 = {}, {}, {}
def _mk(dct, *spec):
    off = 0
    for name, width in spec:
        dct[name] = (off, off + width)
        off += width
    return off
_NW8 = _mk(_W8, ("w1p", 32), ("wc1b", 64), ("wc1b2p", 64), ("pad8", 352))
_NWB = _mk(_WBF, ("w2b", 64), ("wfcb", 4), ("wc2a", 32), ("wc2b", 32),
           ("kmask", 4), ("bmask", 128))
_NW32 = _mk(_W32, ("smsk2", 2), ("smap2", 4),
            ("b1r", 1), ("b2r", 1), ("bc1r", 1))


def _pack_consts(W1, b1, W2, b2, Wfc, Wc1, bc1, Wc2):
    import ml_dtypes
    w8 = np.zeros((128, _NW8), np.float32)
    wb = np.zeros((128, _NWB), np.float32)
    w3 = np.zeros((128, _NW32), np.float32)

    lo = _W8["w1p"][0]
    w8[:, lo:lo + 16] = W1
    lo = _W8["wc1b"][0]
    for a in range(4):
        w8[32 * a:32 * a + 32, lo + 16 * a:lo + 16 * a + 16] = Wc1
    lo = _W8["wc1b2p"][0]
    for s in range(2):          # chunk parity within a pair
        for t in range(2):      # k = 8 + t
            r = 64 * s + 32 * t
            w8[r:r + 32, lo + 32 * s + 16 * t:lo + 32 * s + 16 * t + 16] = Wc1

    lo = _WBF["w2b"][0]
    for j in range(4):
        wb[32 * j:32 * j + 16, lo + 16 * j:lo + 16 * j + 16] = W2
    lo = _WBF["wfcb"][0]
    for j in range(4):
        wb[16 * j:16 * j + 16, lo + j] = Wfc[:, 0]
    lo = _WBF["wc2a"][0]
    for k in range(8):
        wb[16 * k:16 * k + 16, lo + k] = Wc2[:, 0]
    lo = _WBF["wc2b"][0]
    for j in range(4):
        for t in range(2):
            r = 32 * j + 16 * t
            wb[r:r + 16, lo + 8 + t] = Wc2[:, 0]
    lo = _WBF["kmask"][0]
    for j in range(4):
        wb[32 * j:32 * j + 10, lo + j] = 1.0

    # finalize consts live on partitions 64-67 (where l4p/el/m4 sit)
    lo = _WBF["bmask"][0]
    for j in range(4):
        wb[64 + j, lo + 32 * j:lo + 32 * j + 10] = 1.0
    lo = _W32["smsk2"][0]
    for j in range(4):
        w3[64 + j, lo + j // 2] = 1.0
    lo = _W32["smap2"][0]
    for j in range(4):
        w3[j // 2, lo + j] = 1.0
    lo = _W32["b1r"][0]
    for j in range(4):
        w3[32 * j:32 * j + 16, lo] = b1
    lo = _W32["b2r"][0]
    for j in range(4):
        w3[16 * j:16 * j + 16, lo] = b2
    lo = _W32["bc1r"][0]
    for s in range(8):
        w3[16 * s:16 * s + 16, lo] = bc1
    return (w8.astype(ml_dtypes.float8_e4m3), wb.astype(ml_dtypes.bfloat16),
            w3)


_CACHED = None


def _build():
    """Build + bacc-compile the per-core Bass graph (same on all 8 cores)."""
    from contextlib import ExitStack

    import concourse.tile as tile
    from concourse import bacc, mybir

    f32 = mybir.dt.float32
    bf16 = mybir.dt.bfloat16
    f8 = mybir.dt.float8e4
    AF = mybir.ActivationFunctionType
    ALU = mybir.AluOpType

    import concourse.bacc as bacc_mod
    _orig_gat = bacc_mod.get_activation_tables

    def _gat_one_set(arch):
        t = _orig_gat(arch)
        if "natural_log_exp_and_others" not in t:
            return t
        return {k: (v if k == "natural_log_exp_and_others" else set())
                for k, v in t.items()}

    bacc_mod.get_activation_tables = _gat_one_set

    nc = bacc.Bacc("TRN2", target_bir_lowering=False, debug=False,
                   num_devices=NCORES)

    xa_p = nc.dram_tensor("xa", [128, NGRP * GW], f8, kind="ExternalInput").ap()
    w8_p = nc.dram_tensor("w8", [128, _NW8], f8, kind="ExternalInput").ap()
    wb_p = nc.dram_tensor("wb", [128, _NWB], bf16, kind="ExternalInput").ap()
    w3_p = nc.dram_tensor("w3", [128, _NW32], f32, kind="ExternalInput").ap()
    out_p = nc.dram_tensor("out", [128, NGRP * CH], bf16,
                       kind="ExternalOutput").ap()

    with tile.TileContext(nc) as tc, ExitStack() as ctx:
        wpool = ctx.enter_context(tc.tile_pool(name="wc", bufs=1))
        wt8 = wpool.tile([128, _NW8], f8, tag="wt8")
        nc.sync.dma_start(out=wt8[:], in_=w8_p[:])
        wtb = wpool.tile([128, _NWB], bf16, tag="wtb")
        wt3 = wpool.tile([128, _NW32], f32, tag="wt3")

        def w8s(name, rows=128):
            lo, hi = _W8[name]
            return wt8[0:rows, lo:hi]

        def wbs(name, rows=128):
            lo, hi = _WBF[name]
            return wtb[0:rows, lo:hi]

        def w3s(name, rows=128):
            lo, hi = _W32[name]
            return wt3[0:rows, lo:hi]

        def w3hi(name):
            lo, hi = _W32[name]
            return wt3[64:68, lo:hi]

        sb = ctx.enter_context(tc.tile_pool(name="sb", bufs=2))
        ldp = ctx.enter_context(tc.tile_pool(name="ld", bufs=3))
        elp = ctx.enter_context(tc.tile_pool(name="el", bufs=NGRP))
        eyp = ctx.enter_context(tc.tile_pool(name="ey", bufs=NGRP))
        dvp = ctx.enter_context(tc.tile_pool(name="dv", bufs=NGRP))
        accp = ctx.enter_context(tc.tile_pool(name="acc", bufs=1))
        m4p = ctx.enter_context(tc.tile_pool(name="m4p", bufs=NGRP))
        outp = ctx.enter_context(tc.tile_pool(name="outs", bufs=3))
        el_tiles, ey_tiles, m4_tiles = [], [], []
        accs = accp.tile([128, NGRP], f32)

        with tc.tile_pool(name="h1pp", bufs=1, space="PSUM") as h1pp, \
             tc.tile_pool(name="h2lp", bufs=1, space="PSUM") as h2lp, \
             tc.tile_pool(name="h01pp", bufs=2, space="PSUM") as h01pp, \
             tc.tile_pool(name="h2cpp", bufs=1, space="PSUM") as h2cpp, \
             tc.tile_pool(name="ypp", bufs=1, space="PSUM") as ypp, \
             tc.tile_pool(name="d4pp", bufs=1, space="PSUM") as d4pp, \
             tc.tile_pool(name="finp", bufs=1, space="PSUM") as finp:
            ft = finp.tile([128, 512], f32, tag="fin", name="fin")
            sp = ft[0:2, 504:505]
            s4p = ft[64:68, 508:509]
            for c in range(NGRP):
                blob = ldp.tile([128, GW], f8, tag="blob", name="blob")
                nc.sync.dma_start(out=blob[:, 0:4000],
                                  in_=xa_p[:, GW * c:GW * c + 4000])
                nc.sync.dma_start(out=blob[:, 4000:GW],
                                  in_=xa_p[:, GW * c + 4000:GW * (c + 1)])
                xg2 = blob[:, 0:2000]
                c0 = blob[:, 2000:4000]
                c1 = blob[:, 4000:6000]
                c2t = blob[:, 6000:7000]
                if c == 0:
                    nc.sync.dma_start(out=wt3[:], in_=w3_p[:])
                    nc.sync.dma_start(out=wtb[:], in_=wb_p[:])
                    # pre-load the ACT spline table set with no DMA dep
                    warm = sb.tile([1, 2], f32, tag="warm")
                    nc.vector.memset(warm[:, 0:1], 0.0)
                    nc.scalar.activation(warm[:, 1:2], warm[:, 0:1], AF.Exp)

                # --- node path ---
                h1p = h1pp.tile([128, 512], f32, tag="h1p", name="h1p")[:, 0:CH]
                for j in range(GRP):
                    nc.tensor.matmul(h1p[32 * j:32 * j + 32, :],
                                     lhsT=w8s("w1p"),
                                     rhs=xg2[:, CH * j:CH * (j + 1)],
                                     start=True, stop=True,
                                     tile_position=(0, 32 * j))
                h1s = sb.tile([128, CH], bf16, tag="h1s")
                nc.scalar.activation(h1s[:], h1p[:], AF.Relu, bias=w3s("b1r"))
                h2lt = h2lp.tile([128, 512], f32, tag="h2l", name="h2l")
                h2l = h2lt[0:64, 0:CH]
                nc.tensor.matmul(h2l, lhsT=wbs("w2b"), rhs=h1s[:],
                                 start=True, stop=True)
                h2s = sb.tile([64, CH], bf16, tag="h2s")
                nc.scalar.activation(h2s[:], h2l, AF.Relu,
                                     bias=w3s("b2r", 64))
                l4p = h2lt[64:68, 0:CH]
                nc.tensor.matmul(l4p, lhsT=wbs("wfcb", 64),
                                 rhs=h2s[:], start=True, stop=True,
                                 tile_position=(0, 64),
                                 skip_group_check=True)
                el = elp.tile([128, CH], bf16, tag="el")
                nc.scalar.activation(el[64:68, :], l4p, AF.Exp,
                                     accum_out=accs[64:68, c:c + 1])
                el_tiles.append(el)
                # node-softmax denominator accumulation (graph = j//2)
                nc.tensor.matmul(sp, lhsT=w3hi("smsk2"),
                                 rhs=accs[64:68, c:c + 1],
                                 start=(c == 0), stop=(c == NGRP - 1),
                                 tile_position=(64, 0),
                                 skip_group_check=True)

                # --- col path (softmax-independent part) ---
                h2cp = h2cpp.tile([128, 512], f32, tag="h2cp",
                                  name="h2cp")[:, 0:CH]
                for p in range(2):
                    nc.tensor.matmul(h2cp[64 * p:64 * p + 64, :],
                                     lhsT=w8s("wc1b2p"),
                                     rhs=c2t[:, CH * p:CH * (p + 1)],
                                     start=True, stop=True,
                                     tile_position=(0, 64 * p))
                h01s_tiles = []
                for j in range(GRP):
                    cs = slice(CH * j, CH * (j + 1))
                    h01p = h01pp.tile([128, 512], f32, tag="h01p",
                                      name="h01p")[:, 0:CH]
                    nc.tensor.matmul(h01p[0:64, :], lhsT=w8s("wc1b"),
                                     rhs=c0[:, cs], start=True, stop=True)
                    nc.tensor.matmul(h01p[64:128, :], lhsT=w8s("wc1b"),
                                     rhs=c1[:, cs], start=True, stop=True,
                                     tile_position=(0, 64))
                    h01s = sb.tile([128, CH], bf16, tag=f"h01s{j % 2}")
                    if j == 3:
                        nc.scalar.activation(h01s[:], h01p[:], AF.Relu,
                                             bias=w3s("bc1r"))
                    else:
                        nc.vector.tensor_scalar(h01s[:], h01p[:], w3s("bc1r"),
                                                0.0, ALU.add, ALU.max)
                    h01s_tiles.append(h01s)
                h2cs = sb.tile([128, CH], bf16, tag="h2cs")
                nc.scalar.activation(h2cs[:], h2cp[:], AF.Relu,
                                     bias=w3s("bc1r"))
                yp = ypp.tile([128, 512], f32, tag="yp", name="yp")[:, 0:CH]
                for j in range(GRP):
                    nc.tensor.matmul(yp[32 * j:32 * j + 32, :],
                                     lhsT=wbs("wc2a"), rhs=h01s_tiles[j][:],
                                     start=True, stop=False,
                                     skip_group_check=True,
                                     tile_position=(0, 32 * j))
                for j in range(GRP):
                    nc.tensor.matmul(yp[32 * j:32 * j + 32, :],
                                     lhsT=wtb[32 * j:32 * j + 32,
                                              slice(*_WBF["wc2b"])],
                                     rhs=h2cs[32 * j:32 * j + 32, :],
                                     start=False, stop=True,
                                     skip_group_check=True,
                                     tile_position=(32 * j, 32 * j))
                ey = eyp.tile([128, CH], bf16, tag="ey")
                nc.scalar.activation(ey[:], yp[:], AF.Exp)
                ey_tiles.append(ey)
                d4p = d4pp.tile([128, 512], f32, tag="d4p",
                                name="d4p")[64:68, 0:CH]
                nc.tensor.matmul(d4p, lhsT=wbs("kmask"), rhs=ey[:],
                                 start=True, stop=True,
                                 tile_position=(0, 64),
                                 skip_group_check=True)
                lnD = sb.tile([128, CH], f32, tag="lnD")
                nc.scalar.activation(lnD[64:68, :], d4p, AF.Ln)
                dinv = dvp.tile([128, CH], f32, tag="dinv")
                nc.scalar.activation(dinv[64:68, :], lnD[64:68, :], AF.Exp,
                                     scale=-1.0)
                m4 = m4p.tile([128, CH], bf16, tag="m4")
                nc.gpsimd.tensor_mul(m4[64:68, :], el[64:68, :],
                                     dinv[64:68, :])
                m4_tiles.append(m4)

            # ---------- finalize: softmax normalization + output ------
            sinv = sb.tile([2, 1], f32, tag="sinv")
            nc.vector.reciprocal(sinv[:], sp)
            nc.tensor.matmul(s4p, lhsT=w3s("smap2", 2), rhs=sinv[:],
                             start=True, stop=True, tile_position=(0, 64),
                             skip_group_check=True)
            s4s = sb.tile([128, 1], f32, tag="s4s")
            nc.vector.tensor_copy(s4s[64:68, :], s4p)
            bstb = sb.tile([128, 128], bf16, tag="bstb")
            lo, hi = _WBF["bmask"]
            nc.vector.tensor_scalar_mul(bstb[64:68, :], wtb[64:68, lo:lo + 128],
                                        s4s[64:68, :])
            for c in range(NGRP):
                mbp = ft[:, 0:CH]
                nc.tensor.matmul(mbp, lhsT=bstb[64:68, :],
                                 rhs=m4_tiles[c][64:68, :],
                                 start=True, stop=True,
                                 tile_position=(64, 0),
                                 skip_group_check=True)
                ot = outp.tile([128, CH], bf16, tag="ot")
                nc.vector.tensor_mul(ot[:], ey_tiles[c][:], mbp)
                nc.sync.dma_start(out=out_p[:, CH * c:CH * (c + 1)],
                                  in_=ot[:])

    nc.compile()
    bacc_mod.get_activation_tables = _orig_gat
    return nc


def _get_compiled():
    global _CACHED
    if _CACHED is None:
        _CACHED = _build()
    return _CACHED


def _prep_inputs(node_features, col_features, W1, b1, W2, b2, Wfc,
                 Wc1, bc1, Wc2):
    import ml_dtypes
    f8 = ml_dtypes.float8_e4m3
    nf = np.asarray(node_features, np.float32)
    cf = np.asarray(col_features, np.float32)
    xt = np.ascontiguousarray(
        nf.reshape(NCORES, GPC, N, F).transpose(0, 3, 1, 2)
        .reshape(NCORES, F, M)).astype(f8)
    ctf = np.ascontiguousarray(
        cf.reshape(NCORES, GPC, N, K, FC).transpose(0, 3, 4, 1, 2)
        .reshape(NCORES, K * FC, M)).astype(f8)
    # pack per-group blobs: [xg2 | c0 | c1 | c2t]; chunk at row j is 5j+g
    xa = np.empty((NCORES, 128, NGRP * GW), f8)
    for g in range(NGRP):
        o = GW * g
        for j in range(GRP):
            ch = slice(500 * (5 * j + g), 500 * (5 * j + g) + 500)
            xa[:, :, o + 500 * j:o + 500 * (j + 1)] = xt[:, :, ch]
            xa[:, :, o + 2000 + 500 * j:o + 2500 + 500 * j] = \
                ctf[:, 0:128, ch]
            xa[:, :, o + 4000 + 500 * j:o + 4500 + 500 * j] = \
                ctf[:, 128:256, ch]
        for p in range(2):
            for s in range(2):
                ch = slice(500 * (5 * (2 * p + s) + g),
                           500 * (5 * (2 * p + s) + g) + 500)
                xa[:, 64 * s:64 * s + 64,
                   o + 6000 + 500 * p:o + 6500 + 500 * p] = \
                    ctf[:, 256:320, ch]
    w8, wb, w3 = _pack_consts(
        np.asarray(W1, np.float32), np.asarray(b1, np.float32),
        np.asarray(W2, np.float32), np.asarray(b2, np.float32),
        np.asarray(Wfc, np.float32), np.asarray(Wc1, np.float32),
        np.asarray(bc1, np.float32), np.asarray(Wc2, np.float32))
    return xa, w8, wb, w3


def kernel(node_features, col_features, edge_index=None,
           W1=None, b1=None, W2=None, b2=None, Wfc=None, bfc=None,
           Wc1=None, bc1=None, Wc2=None, bc2=None, **_unused):
    from concourse.bass_utils import run_bass_kernel_spmd

    xa, w8, wb, w3 = _prep_inputs(node_features, col_features,
                                  W1, b1, W2, b2, Wfc, Wc1, bc1, Wc2)
    nc = _get_compiled()
    in_maps = [{"xa": xa[i], "w8": w8, "wb": wb, "w3": w3}
               for i in range(NCORES)]
    res = run_bass_kernel_spmd(nc, in_maps, core_ids=list(range(NCORES)))
    outs = np.stack([np.asarray(res.results[i]["out"], np.float32)
                     for i in range(NCORES)])
    # outs[i][32j+k, 500g+nn] = value for node 500*(5j+g)+nn, class k
    o = outs.reshape(NCORES, 4, 32, NGRP, CH)[:, :, 0:K]   # [i, j, k, g, nn]
    o = o.transpose(0, 1, 3, 4, 2)                         # [i, j, g, nn, k]
    out = o.reshape(NCORES, GPC, N, K).reshape(B, N * K)
    return np.ascontiguousarray(out)
